# revision 4
# baseline (speedup 1.0000x reference)
"""Bass/Tile TRN2 kernel v2 for nn_Network_21131239096982 (gnn_message_passing).

Sharding: per-sample (core c handles sample c%4, full [N,N] pair set) --
communication-free conv layers, ONE AllGather of the final per-sample
features [32,128] before the (redundant) batchnorm head. Replica groups
[[0..3],[4..7]]; cores 4-7 duplicate.

Math restructure vs v1 baseline:
- Radial interp via SATURATING RAMPS: s2_l(u) = V_l[127] + sum_g dV_l[g] *
  clamp(a_g*u + b_g, 0, 1), u = r^2. O'[g,pair] = ramp values, built with
  ONE matmul (K=2: [a;b] x [u;1]) + ONE clamp op per 512-col chunk
  (alternating vector-TS / scalar-ACT-Relu + gpsimd-TS-min to balance
  engines). No per-pair radial-MLP matmul at all:
- ftab is folded INTO the G-stage weights on the host:
  V'_l[g,(j,i)] = (Y0/sqrt(HID)) * sum_h dftab_l[g,h] * rw3_l[h,(i,j)],
  so G2[g,(i,y)] = V'_l^T @ fm costs 32 matmuls/layer and the y-loop
  contracts O' directly against G2: pf[i,x] = sum_y G2[:,y,:]^T O'[:,y*N:].
- Gate = ACT Softplus (scale=5) + one STT (x0.2 x mask/sqrt(N)).
- Head in [x-partitions, (b,feat)-columns] layout: W1 via block-diagonal
  weights from the AllGathered t3 [(b,i), x], BN stats are free-dim
  reductions, BN-apply+LeakyReLU is a single Prelu ACT with per-partition
  scale/bias, W2 after an 8-block transpose. Only act tables used:
  softplus set (conv) + sqrt set (head) = 2 table loads total.
"""

import math

import numpy as np

B, N, EMB, MUL = 4, 128, 32, 32
NB, MAXR = 10, 10.0
HID, BETA = 128, 5.0
MID, OUT = 256, 128
NL = 4
Y0 = 1.0 / (2.0 * math.sqrt(math.pi))
NP = N * N                     # 16384 pairs per core (y outer, x inner)
NCORES = 8
GRID_N = 128                   # nodes; 127 ramps + const row
RMAX = 7.5
SQN = 1.0 / math.sqrt(N)
CH = 512                       # pair columns per O-build chunk
NCH = NP // CH                 # 32

# packgeo (f32, 5 rows) column layout -- tiny, lands first
PG = {"geoY": (0, 128), "geoX": (128, 256)}
PG_COLS = 256

# packa (f32) column layout
PA = {}
_c = 0
for _n, _w in [("mgate", 4), ("gbe", 4), ("epscol", 1), ("ident", 128)]:
    PA[_n] = (_c, _c + _w)
    _c += _w
PA_COLS = _c

# pack2 (bf16, 2 rows) column layout -- skinny constants
P2 = {}
_c = 0
for _n, _w in [("ab", 127), ("onesr", 128), ("b1row", 1024),
               ("b2row", 512)]:
    P2[_n] = (_c, _c + _w)
    _c += _w
P2_COLS = _c

# packb (bf16) column layout
PB = {}
_c = 0
for _n, _w in [("fm0", 128), ("wbd", 1024), ("w2c", 256), ("mcol4", 4)]:
    PB[_n] = (_c, _c + _w)
    _c += _w
PB_COLS = _c

_cached = None


def _build():
    import jax

    jax.devices()  # axon boot
    from concourse import bacc, tile, mybir

    F32 = mybir.dt.float32
    BF16 = mybir.dt.bfloat16
    AF = mybir.ActivationFunctionType
    ALU = mybir.AluOpType
    AXX = mybir.AxisListType.X

    nc = bacc.Bacc("TRN2", debug=False, num_devices=NCORES)

    packgeo_d = nc.dram_tensor("packgeo", [5, PG_COLS], F32,
                               kind="ExternalInput").ap()
    packa_d = nc.dram_tensor("packa", [128, PA_COLS], F32,
                             kind="ExternalInput").ap()
    pack2_d = nc.dram_tensor("pack2", [2, P2_COLS], BF16,
                             kind="ExternalInput").ap()
    packb_d = nc.dram_tensor("packb", [128, PB_COLS], BF16,
                             kind="ExternalInput").ap()
    vt_d = nc.dram_tensor("vt", [32, NL * MUL * GRID_N], BF16,
                          kind="ExternalInput").ap()
    onesd_d = nc.dram_tensor("onesd", [1, NP], BF16,
                             kind="ExternalInput").ap()
    out_d = nc.dram_tensor("out", [B, OUT], F32, kind="ExternalOutput").ap()

    with tile.TileContext(nc) as tc:
        with (
            tc.tile_pool(name="const", bufs=1) as cp,
            tc.tile_pool(name="g2p", bufs=2) as g2p,
            tc.tile_pool(name="fmp", bufs=2) as fmp,
            tc.tile_pool(name="wk", bufs=2) as wk,
            tc.tile_pool(name="hd", bufs=1) as hd,
            tc.tile_pool(name="ps_a", bufs=4, space="PSUM") as pA,
            tc.tile_pool(name="ps_pf", bufs=2, space="PSUM") as pPf,
            tc.tile_pool(name="dram", bufs=1, space="DRAM") as dp,
        ):
            packgeo = cp.tile([5, PG_COLS], F32, name="packgeo_sb")
            nc.sync.dma_start(packgeo[:], packgeo_d[:])
            pack2 = cp.tile([2, P2_COLS], BF16, name="pack2_sb")
            nc.sync.dma_start(pack2[:], pack2_d[:])
            packa = cp.tile([128, PA_COLS], F32, name="packa_sb")
            nc.scalar.dma_start(packa[:], packa_d[:])
            u2 = cp.tile([2, NP], BF16, name="u2")
            nc.gpsimd.dma_start(u2[1:2, :], onesd_d[:])
            oprime = cp.tile([128, NP], BF16, name="oprime")
            nc.scalar.dma_start(oprime[127:128, :], onesd_d[:])
            packb = cp.tile([128, PB_COLS], BF16, name="packb_sb")
            nc.scalar.dma_start(packb[:], packb_d[:])
            vt = cp.tile([32, NL * MUL * GRID_N], BF16, name="vt_sb")
            nc.gpsimd.dma_start(vt[:], vt_d[:])

            def pa(name, rows):
                lo, hi = PA[name]
                return packa[0:rows, lo:hi]

            def pb(name, rows):
                lo, hi = PB[name]
                return packb[0:rows, lo:hi]

            geoY = packgeo[0:5, 0:128]
            geoX = packgeo[0:5, 128:256]
            mgate = pa("mgate", 128)
            gbe, epscol, ident = pa("gbe", 128), pa("epscol", 128), \
                pa("ident", 128)
            ab = pack2[0:2, P2["ab"][0]:P2["ab"][1]]
            onesr = pack2[0:1, P2["onesr"][0]:P2["onesr"][1]]
            b1row = pack2[0:1, P2["b1row"][0]:P2["b1row"][1]]
            b2row = pack2[0:1, P2["b2row"][0]:P2["b2row"][1]]
            fm0 = pb("fm0", 32)
            wbd = pb("wbd", 128)
            w2c = pb("w2c", 128)
            mcol4 = pb("mcol4", 128)

            # ---- single act-table load: set with Ln+Exp+Prelu+Square+... ----
            from concourse.hw_specs import get_activation_tables
            tabs = get_activation_tables(nc.m.arch)
            need = {AF.Ln, AF.Exp, AF.Prelu, AF.Square, AF.Abs, AF.Relu,
                    AF.Copy, AF.Identity}
            set_id = next(i for i, (_, s) in enumerate(tabs.items())
                          if need <= s)
            nc.scalar.add_instruction(mybir.InstLoadActFuncSet(
                act_func_set_id=set_id,
                name=nc.scalar.bass.get_next_instruction_name(),
                engine=mybir.EngineType.Activation))

            # ---- warm the CC stream with dummy AllGathers (overlap) ----
            AGROUPS = [[0, 1, 2, 3], [4, 5, 6, 7]]
            ccwi = dp.tile([MUL, N], BF16, name="ccwi")
            for w in range(2):
                ccwo = dp.tile([B * MUL, N], BF16, name=f"ccwo{w}")
                nc.gpsimd.collective_compute(
                    "AllGather", ALU.bypass, replica_groups=AGROUPS,
                    ins=[ccwi.opt()], outs=[ccwo.opt()])

            # ---- u = r^2 row via DRAM bounce ----
            r2ps = pA.tile([128, 128], F32, name="r2ps", tag="a")
            nc.tensor.matmul(r2ps[:], geoY, geoX, start=True, stop=True)
            u2d = wk.tile([128, 128], BF16, name="u2d", tag="u2d", bufs=1)
            nc.vector.tensor_scalar(u2d[:], r2ps[:], 0.0, None, op0=ALU.max)
            ubounce = dp.tile([128, 128], BF16, name="ubounce")
            nc.sync.dma_start(ubounce[:], u2d[:])
            nc.sync.dma_start(
                u2[0:1, :], ubounce.opt().rearrange("p x -> () (p x)"))

            # ---- G2 builder: G2[g, (i, y)] ----
            def build_g2(l, fm_ap):
                g2 = g2p.tile([128, MUL * N], BF16, name=f"g2_{l}", tag="g2")
                for c in range(8):
                    gps = pA.tile([128, 512], F32, name=f"gps{l}", tag="a")
                    for k in range(4):
                        i = c * 4 + k
                        nc.tensor.matmul(
                            gps[:, k * 128:(k + 1) * 128],
                            vt[:, (l * MUL + i) * GRID_N:
                               (l * MUL + i + 1) * GRID_N],
                            fm_ap, start=True, stop=True)
                    dst = g2[:, c * 512:(c + 1) * 512]
                    if c % 2 == 0:
                        nc.scalar.activation(dst, gps[:], AF.Copy)
                    else:
                        nc.vector.tensor_copy(dst, gps[:])
                return g2

            # G2 for layer 0 first: runs while the u-row bounce is in flight
            g2s = [None] * NL
            g2s[0] = build_g2(0, fm0)

            # ---- O' ramps: one K=2 matmul + one clamp per chunk ----
            for c in range(NCH):
                ops_ = pA.tile([127, CH], F32, name="ops", tag="a")
                nc.tensor.matmul(ops_[:], ab, u2[:, c * CH:(c + 1) * CH],
                                 start=True, stop=True)
                dst = oprime[0:127, c * CH:(c + 1) * CH]
                if c % 3 != 2:
                    tch = wk.tile([127, CH], BF16, name="tch", tag="tch",
                                  bufs=3)
                    nc.scalar.activation(tch[:], ops_[:], AF.Relu)
                    nc.vector.tensor_scalar(dst, tch[:], 1.0, None,
                                            op0=ALU.min)
                else:
                    nc.vector.tensor_scalar(dst, ops_[:], 0.0, 1.0,
                                            op0=ALU.max, op1=ALU.min)

            # ---- conv layers ----
            # y-loop with O' blocks as the stationary operand (full PE
            # cell engagement keeps the HAM clock hot); pf is [x, i].
            agi = None
            for l in range(NL):
                g2v = g2s[l][:].rearrange("g (i y) -> g y i", y=N)
                pf = pPf.tile([N, MUL], F32, name=f"pf{l}", tag="pf")
                for y in range(N):
                    nc.tensor.matmul(
                        pf[:], oprime[:, y * N:(y + 1) * N], g2v[:, y, :],
                        start=(y == 0), stop=(y == N - 1))
                # m*softplus(5t)/5 = relu(m*t) + m*ln(1 + exp(-5|t|))/5
                # with m = mask (*1/sqrt(N) for l<3) folded in per-partition
                mcol = mgate[:, 0:1] if l < NL - 1 else mgate[:, 2:3]
                m5col = mgate[:, 1:2] if l < NL - 1 else mgate[:, 3:4]
                a1 = wk.tile([N, MUL], F32, name=f"a1_{l}", tag="t1", bufs=4)
                nc.scalar.activation(a1[:], pf[:], AF.Abs)
                rl = wk.tile([N, MUL], F32, name=f"rl_{l}", tag="t1", bufs=4)
                nc.scalar.activation(rl[:], pf[:], AF.Relu, scale=mcol)
                nc.scalar.activation(a1[:], a1[:], AF.Exp, scale=-BETA)
                nc.scalar.activation(a1[:], a1[:], AF.Ln, bias=1.0)
                gated = wk.tile([N, MUL], F32, name=f"gt_{l}", tag="t1",
                                bufs=4)
                nc.vector.scalar_tensor_tensor(
                    gated[:], a1[:], m5col, rl[:],
                    op0=ALU.mult, op1=ALU.add)
                # transpose back to [i, x]
                fmT = pPf.tile([MUL, N], F32, name=f"fmT{l}", tag="pf")
                nc.tensor.transpose(fmT[:], gated[:], ident)
                if l + 1 < NL:
                    fmn = fmp.tile([MUL, N], BF16, name=f"fm{l + 1}",
                                   tag="fm")
                    nc.vector.tensor_copy(fmn[:], fmT[:])
                    g2s[l + 1] = build_g2(l + 1, fmn[:])
                    if l == 2:
                        # late CC warm: keep the gather path hot right
                        # before the real AllGather
                        ccwo2 = dp.tile([B * MUL, N], BF16, name="ccwo2")
                        nc.gpsimd.collective_compute(
                            "AllGather", ALU.bypass, replica_groups=AGROUPS,
                            ins=[ccwi.opt()], outs=[ccwo2.opt()])
                else:
                    agi16 = wk.tile([MUL, N], BF16, name="agi16", tag="agi",
                                    bufs=1)
                    nc.vector.tensor_copy(agi16[:], fmT[:])
                    agi = dp.tile([MUL, N], BF16, name="agi")
                    nc.sync.dma_start(agi[:], agi16[:])

            # ---- AllGather of per-sample features ----
            ago = dp.tile([B * MUL, N], BF16, name="ago")
            nc.gpsimd.collective_compute(
                "AllGather", ALU.bypass, replica_groups=AGROUPS,
                ins=[agi.opt()], outs=[ago.opt()])
            t3sb = cp.tile([B * MUL, N], BF16, name="t3sb")
            nc.sync.dma_start(t3sb[:], ago.opt())

            # ---- head ----
            def bn_stats(pstiles, cnt, gcol, becol, tagn):
                mus, sqs = [], []
                for m, pt in enumerate(pstiles):
                    mu_ = hd.tile([128, 1], F32, name=f"mu{tagn}{m}",
                                  tag="r1", bufs=16)
                    nc.vector.reduce_sum(mu_[:], pt[:], axis=AXX)
                    sq_ = hd.tile([128, 1], F32, name=f"sq{tagn}{m}",
                                  tag="r1", bufs=16)
                    sqscr = wk.tile([128, 512], BF16, name=f"sqs{tagn}{m}",
                                    tag="sqscr", bufs=2)
                    nc.scalar.activation(sqscr[:], pt[:], AF.Square,
                                         accum_out=sq_[:])
                    mus.append(mu_)
                    sqs.append(sq_)
                if len(pstiles) == 2:
                    nc.vector.tensor_tensor(mus[0][:], mus[0][:], mus[1][:],
                                            op=ALU.add)
                    nc.vector.tensor_tensor(sqs[0][:], sqs[0][:], sqs[1][:],
                                            op=ALU.add)
                mu, sq = mus[0], sqs[0]   # raw sums S, Q
                # var*cnt = Q - S^2/cnt; inv = exp(-0.5*ln(var+eps))
                var = hd.tile([128, 1], F32, name=f"var{tagn}", tag="r1",
                              bufs=16)
                nc.vector.tensor_tensor(var[:], mu[:], mu[:], op=ALU.mult)
                nc.vector.scalar_tensor_tensor(
                    var[:], var[:], -1.0 / cnt, sq[:],
                    op0=ALU.mult, op1=ALU.add)
                inv = hd.tile([128, 1], F32, name=f"inv{tagn}", tag="r1",
                              bufs=16)
                nc.scalar.activation(inv[:], var[:], AF.Ln, scale=1.0 / cnt,
                                     bias=epscol[:, 0:1])
                nc.scalar.activation(inv[:], inv[:], AF.Exp, scale=-0.5)
                svec = hd.tile([128, 1], F32, name=f"sv{tagn}", tag="r1",
                               bufs=16)
                nc.vector.tensor_tensor(svec[:], inv[:], gcol, op=ALU.mult)
                # tvec = becol - (S/cnt)*svec
                tvec = hd.tile([128, 1], F32, name=f"tv{tagn}", tag="r1",
                               bufs=16)
                nc.vector.tensor_tensor(tvec[:], mu[:], svec[:], op=ALU.mult)
                nc.vector.scalar_tensor_tensor(
                    tvec[:], tvec[:], -1.0 / cnt, becol,
                    op0=ALU.mult, op1=ALU.add)
                return svec, tvec

            # W1 stage: h1T[x, (b,m)] in two 512-col psum tiles
            h1ps = []
            for m in range(2):
                hp = pA.tile([128, 512], F32, name=f"h1ps{m}", tag="a")
                nc.tensor.matmul(hp[:], onesr,
                                 b1row[:, m * 512:(m + 1) * 512],
                                 start=True, stop=False)
                nc.tensor.matmul(hp[:], t3sb[:],
                                 wbd[:, m * 512:(m + 1) * 512],
                                 start=False, stop=True)
                h1ps.append(hp)
            svec1, tvec1 = bn_stats(h1ps, float(B * MID),
                                    gbe[:, 0:1], gbe[:, 1:2], "1")
            h1n = hd.tile([128, B * MID], F32, name="h1n")
            for m in range(2):
                nc.scalar.activation(
                    h1n[:, m * 512:(m + 1) * 512], h1ps[m][:], AF.Prelu,
                    scale=svec1[:, 0:1], bias=tvec1[:, 0:1], alpha=0.2)

            # transpose h1n -> h1nT [(b,mh) m-rows, x]
            h1nT = hd.tile([128, B * MID], BF16, name="h1nT")
            for k in range(8):
                trp = pA.tile([128, 128], F32, name=f"trp{k}", tag="a")
                nc.tensor.transpose(trp[:], h1n[:, k * 128:(k + 1) * 128],
                                    ident)
                dst = h1nT[:, k * 128:(k + 1) * 128]
                if k % 2 == 0:
                    nc.scalar.activation(dst, trp[:], AF.Copy)
                else:
                    nc.vector.tensor_copy(dst, trp[:])

            # W2 stage: h2T[x, (b,o)]
            h2ps = pA.tile([128, 512], F32, name="h2ps", tag="a")
            nc.tensor.matmul(h2ps[:], onesr, b2row, start=True, stop=False)
            for b in range(B):
                for mh in range(2):
                    nc.tensor.matmul(
                        h2ps[:, b * 128:(b + 1) * 128],
                        h1nT[:, (b * 2 + mh) * 128:(b * 2 + mh + 1) * 128],
                        w2c[:, mh * 128:(mh + 1) * 128],
                        start=False, stop=(mh == 1))
            svec2, tvec2 = bn_stats([h2ps], float(B * 128),
                                    gbe[:, 2:3], gbe[:, 3:4], "2")
            h2n = hd.tile([128, 512], BF16, name="h2n")
            nc.scalar.activation(h2n[:], h2ps[:], AF.Prelu,
                                 scale=svec2[:, 0:1], bias=tvec2[:, 0:1],
                                 alpha=0.2)

            # masked mean pool -> out[b, o]
            poolps = pA.tile([1, 512], F32, name="poolps", tag="a")
            for b in range(B):
                nc.tensor.matmul(poolps[:, b * 128:(b + 1) * 128],
                                 mcol4[:, b:b + 1],
                                 h2n[:, b * 128:(b + 1) * 128],
                                 start=True, stop=True)
            poolsb = wk.tile([1, 512], F32, name="poolsb", tag="pool", bufs=1)
            nc.vector.tensor_copy(poolsb[:], poolps[:])
            nc.sync.dma_start(out_d.rearrange("b o -> () (b o)"), poolsb[:])

    nc.compile()
    return nc


def _ssp_chain(r, rw1, rw2):
    grid = np.linspace(0.0, MAXR, NB)
    step = grid[1] - grid[0]
    x = (r[..., None] - grid) / step
    basis = np.where(np.abs(x) < 1.0, np.cos(0.5 * math.pi * x) ** 2, 0.0)

    def ssp(v):
        return (np.logaddexp(0, BETA * v) - math.log(2.0)) / BETA

    h = ssp(basis @ rw1 / math.sqrt(NB))
    h = ssp(h @ rw2 / math.sqrt(HID))
    return h


def _host_prep(inputs):
    import ml_dtypes
    BF = ml_dtypes.bfloat16
    f32 = np.float32

    f = {k: np.asarray(v) for k, v in inputs.items()}
    geometry = f["geometry"].astype(np.float64)
    features = f["features"].astype(np.int64)
    mask = f["mask"].astype(np.float64)
    emb = f["emb"].astype(np.float64)
    rw1, rw2, rw3 = (f[k].astype(np.float64) for k in ("rw1", "rw2", "rw3"))
    W1, b1 = f["W1"].astype(np.float64), f["b1"].astype(np.float64)
    W2, b2 = f["W2"].astype(np.float64), f["b2"].astype(np.float64)
    g1, be1 = f["g1"].astype(np.float64), f["be1"].astype(np.float64)
    g2, be2 = f["g2"].astype(np.float64), f["be2"].astype(np.float64)

    # ramp coefficients
    rg = np.linspace(0.0, RMAX, GRID_N)
    c = rg ** 2
    aq = np.asarray(1.0 / (c[1:] - c[:-1]), BF).astype(np.float64)
    bq = -aq * c[:-1]
    ab = np.zeros((2, 127))
    ab[0], ab[1] = aq, bq

    # V' fusion: diffs on ramp rows 0..126, const on row 127
    gfac = Y0 / math.sqrt(HID)
    vt = np.zeros((32, NL * MUL * GRID_N))
    for l in range(NL):
        ftab = _ssp_chain(rg, rw1[l], rw2[l])          # [128, HID]
        r3 = rw3[l].reshape(HID, MUL, MUL)             # [h, i, j]
        V = gfac * np.einsum('gh,hij->gji', ftab, r3)  # [g, j, i]
        Vp = np.zeros((GRID_N, MUL, MUL))              # [row, j, i]
        Vp[:127] = V[1:] - V[:-1]
        Vp[127] = V[0]
        for i in range(MUL):
            vt[:, (l * MUL + i) * GRID_N:(l * MUL + i + 1) * GRID_N] = \
                Vp[:, :, i].T                          # [j, g]

    f0_all = emb[features[..., 0]]                     # [B, N, EMB]
    norms = (geometry ** 2).sum(axis=-1)               # [B, N]
    msum = mask.sum(axis=1)                            # [B]

    # shared head constants
    wbd = np.zeros((128, 1024))
    for b in range(B):
        wbd[b * 32:(b + 1) * 32, b * MID:(b + 1) * MID] = W1
    w2c = np.zeros((128, 256))
    for mh in range(2):
        w2c[:, mh * 128:(mh + 1) * 128] = W2[mh * 128:(mh + 1) * 128, :]
    b1row = np.tile(b1, B).reshape(1, B * MID)
    b2row = np.tile(b2, B).reshape(1, B * 128)
    mcol4 = (mask / msum[:, None]).T                   # [N, B]
    gbe = np.stack([g1, be1, g2, be2], axis=1)         # [128, 4]

    pack2_a = np.zeros((2, P2_COLS))

    def set2(name, arr):
        lo, hi = P2[name]
        r, w = arr.shape
        pack2_a[0:r, lo:lo + w] = arr

    set2("ab", ab)
    set2("onesr", np.ones((1, 128)))
    set2("b1row", b1row)
    set2("b2row", b2row)

    packb_shared = np.zeros((128, PB_COLS))

    def setb(name, arr):
        lo, hi = PB[name]
        r, w = arr.shape
        packb_shared[0:r, lo:lo + w] = arr

    setb("wbd", wbd)
    setb("w2c", w2c)
    setb("mcol4", mcol4)

    packa_shared = np.zeros((128, PA_COLS))

    def seta(name, arr, dst=packa_shared):
        lo, hi = PA[name]
        r, w = arr.shape
        dst[0:r, lo:lo + w] = arr

    seta("gbe", gbe)
    seta("epscol", np.full((128, 1), 1e-5))
    seta("ident", np.eye(128))

    vt_bf = np.asarray(vt, BF)
    onesd = np.ones((1, NP), BF)
    pack2_bf = np.asarray(pack2_a, BF)

    in_maps = []
    for core in range(NCORES):
        b = core % B
        pkg = np.zeros((5, PG_COLS))
        pkg[0:3, 0:128] = -2.0 * geometry[b].T
        pkg[3, 0:128] = norms[b]
        pkg[4, 0:128] = 1.0
        pkg[0:3, 128:256] = geometry[b].T
        pkg[3, 128:256] = 1.0
        pkg[4, 128:256] = norms[b]
        fm0 = (f0_all[b] * mask[b][:, None] * SQN).T   # [32, N]
        pka = packa_shared.copy()
        mg = np.stack([mask[b] * SQN, 0.2 * mask[b] * SQN,
                       mask[b], 0.2 * mask[b]], axis=1)
        seta("mgate", mg, pka)
        pkb = packb_shared.copy()
        pkb[0:32, PB["fm0"][0]:PB["fm0"][0] + 128] = fm0
        in_maps.append({
            "packgeo": pkg.astype(f32),
            "packa": pka.astype(f32),
            "pack2": pack2_bf,
            "packb": np.asarray(pkb, BF),
            "vt": vt_bf,
            "onesd": onesd,
        })
    return in_maps


def run(inputs, trace=False):
    global _cached
    from concourse import bass_utils
    if _cached is None:
        _cached = _build()
    nc = _cached
    in_maps = _host_prep(inputs)
    res = bass_utils.run_bass_kernel_spmd(
        nc, in_maps, core_ids=list(range(NCORES)), trace=trace)
    return res


def kernel(**inputs):
    res = run(inputs, trace=False)
    return np.asarray(res.results[0]["out"], dtype=np.float32)


# revision 5
# speedup vs baseline: 1.1098x; 1.1098x over previous
"""Bass/Tile TRN2 kernel v2 for nn_Network_21131239096982 (gnn_message_passing).

Sharding: per-sample (core c handles sample c%4, full [N,N] pair set) --
communication-free conv layers, ONE AllGather of the final per-sample
features [32,128] (bf16) before the redundant batchnorm head. Replica
groups [[0..3],[4..7]]; cores 4-7 duplicate. Three dummy AllGathers (two
early, one after layer 2) absorb the CC barrier / stream warmup so the
real gather runs ~5-13us instead of ~20us.

Math restructure vs the v1 baseline (what made it 1.6x faster):
- Radial interp via SATURATING RAMPS in u=r^2: s2_l(u) = base_l +
  sum_g dV_l[g] * clamp(a_g*u + b_g, 0, 1). O'[g,pair] (127 ramp rows +
  one const row) is built with ONE K=2 matmul ([a;b] x [u;1]) + ONE
  clamp per 512-col chunk, split between vector-TS (psum max/min) and
  scalar-ACT-Relu + vector-TS-min paths to balance engines.
- ftab is folded INTO the G-stage weights on the host:
  V'_l[g,(j,i)] = (Y0/sqrt(HID)) * sum_h dftab_l[g,h] * rw3_l[h,(i,j)],
  so G2[g,(i,y)] = V'_l^T @ fm is 32 matmuls/layer and the y-loop
  contracts O' directly against G2 -- the per-pair radial-MLP matmuls
  (s2 = ftab^T O, ~65k PE columns) are gone entirely.
- y-loop keeps O' blocks [128,128] as the STATIONARY operand: full PE
  cell engagement keeps the HAM clock at 2.4GHz (57ns/matmul pacing);
  pf comes out [x, i] and is transposed back after the gate.
- Gate = relu(m*t) + m*ln(1+exp(-5|t|))/5 with the mask (*1/sqrt(N))
  folded into per-partition ACT scale / STT scalar APs; single act
  table set (natural log + exp) loaded once explicitly.
- Head in [x-partitions, (b,feat)-columns] layout: W1 via block-diagonal
  weights from the AllGathered t3 [(b,i), x], BN stats are free-dim
  reductions (Square-ACT accum_out + vector reduce), BN-apply+LeakyReLU
  is a single Prelu ACT with per-partition scale/bias, W2 after an
  8-block transpose, pool via per-sample mask-column matmuls.
- Constants packed into few DMAs sized so the hot ones (geometry, ramp
  coefs) land first; G2 for layer 0 is emitted before the O'-build so
  the tensor queue works while the u-row DRAM bounce is in flight.
"""

import math

import numpy as np

B, N, EMB, MUL = 4, 128, 32, 32
NB, MAXR = 10, 10.0
HID, BETA = 128, 5.0
MID, OUT = 256, 128
NL = 4
Y0 = 1.0 / (2.0 * math.sqrt(math.pi))
NP = N * N                     # 16384 pairs per core (y outer, x inner)
NCORES = 8
GRID_N = 128                   # nodes; 127 ramps + const row
RMAX = 7.5
SQN = 1.0 / math.sqrt(N)
CH = 512                       # pair columns per O-build chunk
NCH = NP // CH                 # 32

# packgeo (f32, 5 rows) column layout -- tiny, lands first
PG = {"geoY": (0, 128), "geoX": (128, 256)}
PG_COLS = 256

# packa (f32) column layout
PA = {}
_c = 0
for _n, _w in [("mgate", 4), ("gbe", 4), ("epscol", 1), ("ident", 128)]:
    PA[_n] = (_c, _c + _w)
    _c += _w
PA_COLS = _c

# pack2 (bf16, 2 rows) column layout -- skinny constants
P2 = {}
_c = 0
for _n, _w in [("ab", 127), ("onesr", 128), ("b1row", 1024),
               ("b2row", 512)]:
    P2[_n] = (_c, _c + _w)
    _c += _w
P2_COLS = _c

# packb (bf16) column layout
PB = {}
_c = 0
for _n, _w in [("fm0", 128), ("wbd", 1024), ("w2c", 256), ("mcol4", 4)]:
    PB[_n] = (_c, _c + _w)
    _c += _w
PB_COLS = _c

_cached = None


def _build():
    import jax

    jax.devices()  # axon boot
    from concourse import bacc, tile, mybir

    F32 = mybir.dt.float32
    BF16 = mybir.dt.bfloat16
    AF = mybir.ActivationFunctionType
    ALU = mybir.AluOpType
    AXX = mybir.AxisListType.X

    nc = bacc.Bacc("TRN2", debug=False, num_devices=NCORES)

    packgeo_d = nc.dram_tensor("packgeo", [5, PG_COLS], F32,
                               kind="ExternalInput").ap()
    packa_d = nc.dram_tensor("packa", [128, PA_COLS], F32,
                             kind="ExternalInput").ap()
    pack2_d = nc.dram_tensor("pack2", [2, P2_COLS], BF16,
                             kind="ExternalInput").ap()
    packb_d = nc.dram_tensor("packb", [128, PB_COLS], BF16,
                             kind="ExternalInput").ap()
    vt_d = nc.dram_tensor("vt", [32, NL * MUL * GRID_N], BF16,
                          kind="ExternalInput").ap()
    onesd_d = nc.dram_tensor("onesd", [1, NP], BF16,
                             kind="ExternalInput").ap()
    out_d = nc.dram_tensor("out", [B, OUT], F32, kind="ExternalOutput").ap()

    with tile.TileContext(nc) as tc:
        with (
            tc.tile_pool(name="const", bufs=1) as cp,
            tc.tile_pool(name="g2p", bufs=2) as g2p,
            tc.tile_pool(name="fmp", bufs=2) as fmp,
            tc.tile_pool(name="wk", bufs=2) as wk,
            tc.tile_pool(name="hd", bufs=1) as hd,
            tc.tile_pool(name="ps_a", bufs=4, space="PSUM") as pA,
            tc.tile_pool(name="ps_pf", bufs=2, space="PSUM") as pPf,
            tc.tile_pool(name="dram", bufs=1, space="DRAM") as dp,
        ):
            packgeo = cp.tile([5, PG_COLS], F32, name="packgeo_sb")
            nc.sync.dma_start(packgeo[:], packgeo_d[:])
            pack2 = cp.tile([2, P2_COLS], BF16, name="pack2_sb")
            nc.sync.dma_start(pack2[:], pack2_d[:])
            packa = cp.tile([128, PA_COLS], F32, name="packa_sb")
            nc.scalar.dma_start(packa[:], packa_d[:])
            u2 = cp.tile([2, NP], BF16, name="u2")
            nc.gpsimd.dma_start(u2[1:2, :], onesd_d[:])
            oprime = cp.tile([128, NP], BF16, name="oprime")
            nc.scalar.dma_start(oprime[127:128, :], onesd_d[:])
            packb = cp.tile([128, PB_COLS], BF16, name="packb_sb")
            nc.scalar.dma_start(packb[:], packb_d[:])
            vt = cp.tile([32, NL * MUL * GRID_N], BF16, name="vt_sb")
            nc.gpsimd.dma_start(vt[:], vt_d[:])

            def pa(name, rows):
                lo, hi = PA[name]
                return packa[0:rows, lo:hi]

            def pb(name, rows):
                lo, hi = PB[name]
                return packb[0:rows, lo:hi]

            geoY = packgeo[0:5, 0:128]
            geoX = packgeo[0:5, 128:256]
            mgate = pa("mgate", 128)
            gbe, epscol, ident = pa("gbe", 128), pa("epscol", 128), \
                pa("ident", 128)
            ab = pack2[0:2, P2["ab"][0]:P2["ab"][1]]
            onesr = pack2[0:1, P2["onesr"][0]:P2["onesr"][1]]
            b1row = pack2[0:1, P2["b1row"][0]:P2["b1row"][1]]
            b2row = pack2[0:1, P2["b2row"][0]:P2["b2row"][1]]
            fm0 = pb("fm0", 32)
            wbd = pb("wbd", 128)
            w2c = pb("w2c", 128)
            mcol4 = pb("mcol4", 128)

            # ---- single act-table load: set with Ln+Exp+Prelu+Square+... ----
            from concourse.hw_specs import get_activation_tables
            tabs = get_activation_tables(nc.m.arch)
            need = {AF.Ln, AF.Exp, AF.Prelu, AF.Square, AF.Abs, AF.Relu,
                    AF.Copy, AF.Identity}
            set_id = next(i for i, (_, s) in enumerate(tabs.items())
                          if need <= s)
            nc.scalar.add_instruction(mybir.InstLoadActFuncSet(
                act_func_set_id=set_id,
                name=nc.scalar.bass.get_next_instruction_name(),
                engine=mybir.EngineType.Activation))

            # ---- warm the CC stream with dummy AllGathers (overlap) ----
            AGROUPS = [[0, 1, 2, 3], [4, 5, 6, 7]]
            ccwi = dp.tile([MUL, N], BF16, name="ccwi")
            for w in range(2):
                ccwo = dp.tile([B * MUL, N], BF16, name=f"ccwo{w}")
                nc.gpsimd.collective_compute(
                    "AllGather", ALU.bypass, replica_groups=AGROUPS,
                    ins=[ccwi.opt()], outs=[ccwo.opt()])

            # ---- u = r^2 row via DRAM bounce ----
            r2ps = pA.tile([128, 128], F32, name="r2ps", tag="a")
            nc.tensor.matmul(r2ps[:], geoY, geoX, start=True, stop=True)
            u2d = wk.tile([128, 128], BF16, name="u2d", tag="u2d", bufs=1)
            nc.vector.tensor_scalar(u2d[:], r2ps[:], 0.0, None, op0=ALU.max)
            ubounce = dp.tile([128, 128], BF16, name="ubounce")
            nc.sync.dma_start(ubounce[:], u2d[:])
            nc.sync.dma_start(
                u2[0:1, :], ubounce.opt().rearrange("p x -> () (p x)"))

            # ---- G2 builder: G2[g, (i, y)] ----
            def build_g2(l, fm_ap):
                g2 = g2p.tile([128, MUL * N], BF16, name=f"g2_{l}", tag="g2")
                for c in range(8):
                    gps = pA.tile([128, 512], F32, name=f"gps{l}", tag="a")
                    for k in range(4):
                        i = c * 4 + k
                        nc.tensor.matmul(
                            gps[:, k * 128:(k + 1) * 128],
                            vt[:, (l * MUL + i) * GRID_N:
                               (l * MUL + i + 1) * GRID_N],
                            fm_ap, start=True, stop=True)
                    dst = g2[:, c * 512:(c + 1) * 512]
                    if c % 2 == 0:
                        nc.scalar.activation(dst, gps[:], AF.Copy)
                    else:
                        nc.vector.tensor_copy(dst, gps[:])
                return g2

            # G2 for layer 0 first: runs while the u-row bounce is in flight
            g2s = [None] * NL
            g2s[0] = build_g2(0, fm0)

            # ---- O' ramps: one K=2 matmul + one clamp per chunk ----
            for c in range(NCH):
                ops_ = pA.tile([127, CH], F32, name="ops", tag="a")
                nc.tensor.matmul(ops_[:], ab, u2[:, c * CH:(c + 1) * CH],
                                 start=True, stop=True)
                dst = oprime[0:127, c * CH:(c + 1) * CH]
                if c % 3 != 2:
                    tch = wk.tile([127, CH], BF16, name="tch", tag="tch",
                                  bufs=3)
                    nc.scalar.activation(tch[:], ops_[:], AF.Relu)
                    nc.vector.tensor_scalar(dst, tch[:], 1.0, None,
                                            op0=ALU.min)
                else:
                    nc.vector.tensor_scalar(dst, ops_[:], 0.0, 1.0,
                                            op0=ALU.max, op1=ALU.min)

            # ---- conv layers ----
            # y-loop with O' blocks as the stationary operand (full PE
            # cell engagement keeps the HAM clock hot); pf is [x, i].
            agi = None
            for l in range(NL):
                g2v = g2s[l][:].rearrange("g (i y) -> g y i", y=N)
                pf = pPf.tile([N, MUL], F32, name=f"pf{l}", tag="pf")
                for y in range(N):
                    nc.tensor.matmul(
                        pf[:], oprime[:, y * N:(y + 1) * N], g2v[:, y, :],
                        start=(y == 0), stop=(y == N - 1))
                # m*softplus(5t)/5 = relu(m*t) + m*ln(1 + exp(-5|t|))/5
                # with m = mask (*1/sqrt(N) for l<3) folded in per-partition
                mcol = mgate[:, 0:1] if l < NL - 1 else mgate[:, 2:3]
                m5col = mgate[:, 1:2] if l < NL - 1 else mgate[:, 3:4]
                a1 = wk.tile([N, MUL], F32, name=f"a1_{l}", tag="t1", bufs=4)
                nc.scalar.activation(a1[:], pf[:], AF.Abs)
                rl = wk.tile([N, MUL], F32, name=f"rl_{l}", tag="t1", bufs=4)
                nc.scalar.activation(rl[:], pf[:], AF.Relu, scale=mcol)
                nc.scalar.activation(a1[:], a1[:], AF.Exp, scale=-BETA)
                nc.scalar.activation(a1[:], a1[:], AF.Ln, bias=1.0)
                gated = wk.tile([N, MUL], F32, name=f"gt_{l}", tag="t1",
                                bufs=4)
                nc.vector.scalar_tensor_tensor(
                    gated[:], a1[:], m5col, rl[:],
                    op0=ALU.mult, op1=ALU.add)
                # transpose back to [i, x]
                fmT = pPf.tile([MUL, N], F32, name=f"fmT{l}", tag="pf")
                nc.tensor.transpose(fmT[:], gated[:], ident)
                if l + 1 < NL:
                    fmn = fmp.tile([MUL, N], BF16, name=f"fm{l + 1}",
                                   tag="fm")
                    nc.vector.tensor_copy(fmn[:], fmT[:])
                    g2s[l + 1] = build_g2(l + 1, fmn[:])
                    if l == 2:
                        # late CC warm: keep the gather path hot right
                        # before the real AllGather
                        ccwo2 = dp.tile([B * MUL, N], BF16, name="ccwo2")
                        nc.gpsimd.collective_compute(
                            "AllGather", ALU.bypass, replica_groups=AGROUPS,
                            ins=[ccwi.opt()], outs=[ccwo2.opt()])
                else:
                    agi16 = wk.tile([MUL, N], BF16, name="agi16", tag="agi",
                                    bufs=1)
                    nc.vector.tensor_copy(agi16[:], fmT[:])
                    agi = dp.tile([MUL, N], BF16, name="agi")
                    nc.sync.dma_start(agi[:], agi16[:])

            # ---- AllGather of per-sample features ----
            ago = dp.tile([B * MUL, N], BF16, name="ago")
            nc.gpsimd.collective_compute(
                "AllGather", ALU.bypass, replica_groups=AGROUPS,
                ins=[agi.opt()], outs=[ago.opt()])
            t3sb = cp.tile([B * MUL, N], BF16, name="t3sb")
            nc.sync.dma_start(t3sb[:], ago.opt())

            # ---- head ----
            def bn_stats(pstiles, cnt, gcol, becol, tagn):
                mus, sqs = [], []
                for m, pt in enumerate(pstiles):
                    mu_ = hd.tile([128, 1], F32, name=f"mu{tagn}{m}",
                                  tag="r1", bufs=16)
                    nc.vector.reduce_sum(mu_[:], pt[:], axis=AXX)
                    sq_ = hd.tile([128, 1], F32, name=f"sq{tagn}{m}",
                                  tag="r1", bufs=16)
                    sqscr = wk.tile([128, 512], BF16, name=f"sqs{tagn}{m}",
                                    tag="sqscr", bufs=2)
                    nc.scalar.activation(sqscr[:], pt[:], AF.Square,
                                         accum_out=sq_[:])
                    mus.append(mu_)
                    sqs.append(sq_)
                if len(pstiles) == 2:
                    nc.vector.tensor_tensor(mus[0][:], mus[0][:], mus[1][:],
                                            op=ALU.add)
                    nc.vector.tensor_tensor(sqs[0][:], sqs[0][:], sqs[1][:],
                                            op=ALU.add)
                mu, sq = mus[0], sqs[0]   # raw sums S, Q
                # var*cnt = Q - S^2/cnt; inv = exp(-0.5*ln(var+eps))
                var = hd.tile([128, 1], F32, name=f"var{tagn}", tag="r1",
                              bufs=16)
                nc.vector.tensor_tensor(var[:], mu[:], mu[:], op=ALU.mult)
                nc.vector.scalar_tensor_tensor(
                    var[:], var[:], -1.0 / cnt, sq[:],
                    op0=ALU.mult, op1=ALU.add)
                inv = hd.tile([128, 1], F32, name=f"inv{tagn}", tag="r1",
                              bufs=16)
                nc.scalar.activation(inv[:], var[:], AF.Ln, scale=1.0 / cnt,
                                     bias=epscol[:, 0:1])
                nc.scalar.activation(inv[:], inv[:], AF.Exp, scale=-0.5)
                svec = hd.tile([128, 1], F32, name=f"sv{tagn}", tag="r1",
                               bufs=16)
                nc.vector.tensor_tensor(svec[:], inv[:], gcol, op=ALU.mult)
                # tvec = becol - (S/cnt)*svec
                tvec = hd.tile([128, 1], F32, name=f"tv{tagn}", tag="r1",
                               bufs=16)
                nc.vector.tensor_tensor(tvec[:], mu[:], svec[:], op=ALU.mult)
                nc.vector.scalar_tensor_tensor(
                    tvec[:], tvec[:], -1.0 / cnt, becol,
                    op0=ALU.mult, op1=ALU.add)
                return svec, tvec

            # W1 stage: h1T[x, (b,m)] in two 512-col psum tiles
            h1ps = []
            for m in range(2):
                hp = pA.tile([128, 512], F32, name=f"h1ps{m}", tag="a")
                nc.tensor.matmul(hp[:], onesr,
                                 b1row[:, m * 512:(m + 1) * 512],
                                 start=True, stop=False)
                nc.tensor.matmul(hp[:], t3sb[:],
                                 wbd[:, m * 512:(m + 1) * 512],
                                 start=False, stop=True)
                h1ps.append(hp)
            svec1, tvec1 = bn_stats(h1ps, float(B * MID),
                                    gbe[:, 0:1], gbe[:, 1:2], "1")
            h1n = hd.tile([128, B * MID], F32, name="h1n")
            for m in range(2):
                nc.scalar.activation(
                    h1n[:, m * 512:(m + 1) * 512], h1ps[m][:], AF.Prelu,
                    scale=svec1[:, 0:1], bias=tvec1[:, 0:1], alpha=0.2)

            # transpose h1n -> h1nT [(b,mh) m-rows, x]
            h1nT = hd.tile([128, B * MID], BF16, name="h1nT")
            for k in range(8):
                trp = pA.tile([128, 128], F32, name=f"trp{k}", tag="a")
                nc.tensor.transpose(trp[:], h1n[:, k * 128:(k + 1) * 128],
                                    ident)
                dst = h1nT[:, k * 128:(k + 1) * 128]
                if k % 2 == 0:
                    nc.scalar.activation(dst, trp[:], AF.Copy)
                else:
                    nc.vector.tensor_copy(dst, trp[:])

            # W2 stage: h2T[x, (b,o)]
            h2ps = pA.tile([128, 512], F32, name="h2ps", tag="a")
            nc.tensor.matmul(h2ps[:], onesr, b2row, start=True, stop=False)
            for b in range(B):
                for mh in range(2):
                    nc.tensor.matmul(
                        h2ps[:, b * 128:(b + 1) * 128],
                        h1nT[:, (b * 2 + mh) * 128:(b * 2 + mh + 1) * 128],
                        w2c[:, mh * 128:(mh + 1) * 128],
                        start=False, stop=(mh == 1))
            svec2, tvec2 = bn_stats([h2ps], float(B * 128),
                                    gbe[:, 2:3], gbe[:, 3:4], "2")
            h2n = hd.tile([128, 512], BF16, name="h2n")
            nc.scalar.activation(h2n[:], h2ps[:], AF.Prelu,
                                 scale=svec2[:, 0:1], bias=tvec2[:, 0:1],
                                 alpha=0.2)

            # masked mean pool -> out[b, o]
            poolps = pA.tile([1, 512], F32, name="poolps", tag="a")
            for b in range(B):
                nc.tensor.matmul(poolps[:, b * 128:(b + 1) * 128],
                                 mcol4[:, b:b + 1],
                                 h2n[:, b * 128:(b + 1) * 128],
                                 start=True, stop=True)
            poolsb = wk.tile([1, 512], F32, name="poolsb", tag="pool", bufs=1)
            nc.vector.tensor_copy(poolsb[:], poolps[:])
            nc.sync.dma_start(out_d.rearrange("b o -> () (b o)"), poolsb[:])

    nc.compile()
    return nc


def _ssp_chain(r, rw1, rw2):
    grid = np.linspace(0.0, MAXR, NB)
    step = grid[1] - grid[0]
    x = (r[..., None] - grid) / step
    basis = np.where(np.abs(x) < 1.0, np.cos(0.5 * math.pi * x) ** 2, 0.0)

    def ssp(v):
        return (np.logaddexp(0, BETA * v) - math.log(2.0)) / BETA

    h = ssp(basis @ rw1 / math.sqrt(NB))
    h = ssp(h @ rw2 / math.sqrt(HID))
    return h


def _host_prep(inputs):
    import ml_dtypes
    BF = ml_dtypes.bfloat16
    f32 = np.float32

    f = {k: np.asarray(v) for k, v in inputs.items()}
    geometry = f["geometry"].astype(np.float64)
    features = f["features"].astype(np.int64)
    mask = f["mask"].astype(np.float64)
    emb = f["emb"].astype(np.float64)
    rw1, rw2, rw3 = (f[k].astype(np.float64) for k in ("rw1", "rw2", "rw3"))
    W1, b1 = f["W1"].astype(np.float64), f["b1"].astype(np.float64)
    W2, b2 = f["W2"].astype(np.float64), f["b2"].astype(np.float64)
    g1, be1 = f["g1"].astype(np.float64), f["be1"].astype(np.float64)
    g2, be2 = f["g2"].astype(np.float64), f["be2"].astype(np.float64)

    # ramp coefficients
    rg = np.linspace(0.0, RMAX, GRID_N)
    c = rg ** 2
    aq = np.asarray(1.0 / (c[1:] - c[:-1]), BF).astype(np.float64)
    bq = -aq * c[:-1]
    ab = np.zeros((2, 127))
    ab[0], ab[1] = aq, bq

    # V' fusion: diffs on ramp rows 0..126, const on row 127
    gfac = Y0 / math.sqrt(HID)
    vt = np.zeros((32, NL * MUL * GRID_N))
    for l in range(NL):
        ftab = _ssp_chain(rg, rw1[l], rw2[l])          # [128, HID]
        r3 = rw3[l].reshape(HID, MUL, MUL)             # [h, i, j]
        V = gfac * np.einsum('gh,hij->gji', ftab, r3)  # [g, j, i]
        Vp = np.zeros((GRID_N, MUL, MUL))              # [row, j, i]
        Vp[:127] = V[1:] - V[:-1]
        Vp[127] = V[0]
        for i in range(MUL):
            vt[:, (l * MUL + i) * GRID_N:(l * MUL + i + 1) * GRID_N] = \
                Vp[:, :, i].T                          # [j, g]

    f0_all = emb[features[..., 0]]                     # [B, N, EMB]
    norms = (geometry ** 2).sum(axis=-1)               # [B, N]
    msum = mask.sum(axis=1)                            # [B]

    # shared head constants
    wbd = np.zeros((128, 1024))
    for b in range(B):
        wbd[b * 32:(b + 1) * 32, b * MID:(b + 1) * MID] = W1
    w2c = np.zeros((128, 256))
    for mh in range(2):
        w2c[:, mh * 128:(mh + 1) * 128] = W2[mh * 128:(mh + 1) * 128, :]
    b1row = np.tile(b1, B).reshape(1, B * MID)
    b2row = np.tile(b2, B).reshape(1, B * 128)
    mcol4 = (mask / msum[:, None]).T                   # [N, B]
    gbe = np.stack([g1, be1, g2, be2], axis=1)         # [128, 4]

    pack2_a = np.zeros((2, P2_COLS))

    def set2(name, arr):
        lo, hi = P2[name]
        r, w = arr.shape
        pack2_a[0:r, lo:lo + w] = arr

    set2("ab", ab)
    set2("onesr", np.ones((1, 128)))
    set2("b1row", b1row)
    set2("b2row", b2row)

    packb_shared = np.zeros((128, PB_COLS))

    def setb(name, arr):
        lo, hi = PB[name]
        r, w = arr.shape
        packb_shared[0:r, lo:lo + w] = arr

    setb("wbd", wbd)
    setb("w2c", w2c)
    setb("mcol4", mcol4)

    packa_shared = np.zeros((128, PA_COLS))

    def seta(name, arr, dst=packa_shared):
        lo, hi = PA[name]
        r, w = arr.shape
        dst[0:r, lo:lo + w] = arr

    seta("gbe", gbe)
    seta("epscol", np.full((128, 1), 1e-5))
    seta("ident", np.eye(128))

    vt_bf = np.asarray(vt, BF)
    onesd = np.ones((1, NP), BF)
    pack2_bf = np.asarray(pack2_a, BF)

    in_maps = []
    for core in range(NCORES):
        b = core % B
        pkg = np.zeros((5, PG_COLS))
        pkg[0:3, 0:128] = -2.0 * geometry[b].T
        pkg[3, 0:128] = norms[b]
        pkg[4, 0:128] = 1.0
        pkg[0:3, 128:256] = geometry[b].T
        pkg[3, 128:256] = 1.0
        pkg[4, 128:256] = norms[b]
        fm0 = (f0_all[b] * mask[b][:, None] * SQN).T   # [32, N]
        pka = packa_shared.copy()
        mg = np.stack([mask[b] * SQN, 0.2 * mask[b] * SQN,
                       mask[b], 0.2 * mask[b]], axis=1)
        seta("mgate", mg, pka)
        pkb = packb_shared.copy()
        pkb[0:32, PB["fm0"][0]:PB["fm0"][0] + 128] = fm0
        in_maps.append({
            "packgeo": pkg.astype(f32),
            "packa": pka.astype(f32),
            "pack2": pack2_bf,
            "packb": np.asarray(pkb, BF),
            "vt": vt_bf,
            "onesd": onesd,
        })
    return in_maps


def run(inputs, trace=False):
    global _cached
    from concourse import bass_utils
    if _cached is None:
        _cached = _build()
    nc = _cached
    in_maps = _host_prep(inputs)
    res = bass_utils.run_bass_kernel_spmd(
        nc, in_maps, core_ids=list(range(NCORES)), trace=trace)
    return res


def kernel(**inputs):
    res = run(inputs, trace=False)
    return np.asarray(res.results[0]["out"], dtype=np.float32)
